# revision 43
# baseline (speedup 1.0000x reference)
"""Trainium2 Bass kernel for masked multi-head attention (B=4, S=2048, D=512, H=8, dk=64).

Sharding: every batch is split into 8 uniform slices = (head-pair p, k-half kh);
core c = (p = c%4, kh = c//4) runs FOUR slots (one per batch), each with that
batch's EXACT (ceil(nkt/2) k-tiles, nq) shape.  All cores run an identical
instruction stream (SPMD) but per-core work is near-perfectly balanced because
every core owns exactly 1/8 of every batch's score columns.

Per-core kernel tricks:
  - mask-free softmax: the host zeroes k/v columns >= V_len (and k-tile pads),
    so masked scores are exactly 0 -> exp=1, and the matching vp rows AND the
    appended ones-column (host-supplied mask, not memset) are 0, so masked
    positions contribute 0 to both numerator and denominator.  No exp bias.
  - scores computed transposed (S^T[k, q]) feeding AV directly; the two heads
    of a pair run as concurrent K=64 matmuls in PE row-groups 0-1 / 2-3
    (tile_position auto-derived from kp row offsets 0 / 64).
  - V gets a masked ones-column so the AV matmul also emits denominators.
  - k-half partial sums (numerator rows + denominator row) are combined on the
    host before the divide; host also applies the q mask.
  - inputs are host-prefolded to the exact SBUF layout [128, 4*n] so every
    input DMA is 128 fully-contiguous descriptors (HWDGE via the sync queue);
    outputs drain per q-window on the same ring after inputs complete.
"""

import numpy as np
import ml_dtypes

import concourse.bass as bass
import concourse.tile as tile
from concourse import bacc
from concourse import mybir
from concourse import bass2jax

B, S, D = 4, 2048, 512
HEADS, DK = 8, 64
P = 128
DT = D // P  # 4 contraction chunks over D
E = DK + 1   # 64 output dims + denominator row

BF16 = mybir.dt.bfloat16
F32 = mybir.dt.float32

_COMPILE_CACHE = {}


def _cuts(n, w=512):
    return [(s, min(w, n - s)) for s in range(0, n, w)]


def build_bass(key):
    """key: tuple of (KH, nq) per slot, compute order (ascending work)."""
    nc = bacc.Bacc(None, target_bir_lowering=False, debug=False)
    slots = list(key)
    KTS = sum(kh for kh, _ in slots)
    NQS = sum(nq for _, nq in slots)
    NKS = sum(kh * P for kh, _ in slots)
    max_nq = max(nq for _, nq in slots)

    qc = nc.declare_dram_parameter("qc", [P, DT * NQS], BF16, isOutput=False)
    kvc = nc.declare_dram_parameter(
        "kvc", [P, 2 * DT * NKS], BF16, isOutput=False)
    wc = nc.declare_dram_parameter("wc", [P, DT * 3 * P], BF16, isOutput=False)
    om = nc.declare_dram_parameter("om", [P, KTS * 2], F32, isOutput=False)
    out = nc.declare_dram_parameter(
        "out", [len(slots) * 2 * E, max_nq], F32, isOutput=True)

    with tile.TileContext(nc) as tc:
        with (
            tc.tile_pool(name="w", bufs=1) as w_pool,
            tc.tile_pool(name="x", bufs=1) as x_pool,
            tc.tile_pool(name="pr", bufs=1) as pr_pool,
            tc.tile_pool(name="aexp", bufs=3) as a_pool,
            tc.tile_pool(name="osb", bufs=2) as o_pool,
            tc.tile_pool(name="psS", bufs=3, space="PSUM") as psS,
            tc.tile_pool(name="psO", bufs=2, space="PSUM") as psO,
        ):
            w_sb = w_pool.tile([P, DT, 3 * P], BF16, name="w")
            nc.sync.dma_start(
                out=w_sb, in_=wc.rearrange("p (t n) -> p t n", t=DT))
            om_sb = w_pool.tile([P, KTS, 2], F32, name="om")
            nc.sync.dma_start(
                out=om_sb, in_=om.rearrange("p (t n) -> p t n", n=2))

            x_sb = {}
            qoff = koff = 0
            for si, (KH, nq) in enumerate(slots):
                nk = KH * P
                xq = x_pool.tile([P, DT, nq], BF16, tag=f"xq{si}", name=f"xq{si}")
                nc.sync.dma_start(
                    out=xq,
                    in_=qc[:, DT * qoff:DT * (qoff + nq)].rearrange(
                        "p (t n) -> p t n", t=DT))
                xkv = x_pool.tile([P, 2, DT, nk], BF16, tag=f"xkv{si}",
                                  name=f"xkv{si}")
                kvblk = kvc[:, 2 * DT * koff:2 * DT * (koff + nk)].rearrange(
                    "p (u t n) -> p u t n", u=2, t=DT)
                if si == len(slots) - 1 and nk > 512:
                    # last (dominant) slot: split its k/v transfers at the
                    # first 512-projection-cut boundary so its projections
                    # start on first-part arrival instead of waiting for the
                    # whole trailing DMA
                    hk = 512
                    for u in range(2):
                        nc.sync.dma_start(
                            out=xkv[:, u, :, :hk], in_=kvblk[:, u, :, :hk])
                        nc.sync.dma_start(
                            out=xkv[:, u, :, hk:], in_=kvblk[:, u, :, hk:])
                else:
                    nc.sync.dma_start(out=xkv[:, 0], in_=kvblk[:, 0])
                    nc.sync.dma_start(out=xkv[:, 1], in_=kvblk[:, 1])
                x_sb[si] = (xq, xkv[:, 0], xkv[:, 1])
                qoff += nq
                koff += nk

            def ps_tile(width):
                t = psS.tile([P, 1024], F32, tag="s", name="ps")
                return t[:, :width]

            pr = {}

            def emit_proj(si, ktbase):
                KH, nq = slots[si]
                xq, xk, xv = x_sb[si]
                qp = pr_pool.tile([P, nq], BF16, tag=f"qp{si}", name=f"qp{si}")
                for (off, wd) in _cuts(nq):
                    ps = ps_tile(wd)
                    for kd in range(DT):
                        nc.tensor.matmul(
                            ps, lhsT=w_sb[:, kd, 0:P],
                            rhs=xq[:, kd, off:off + wd],
                            start=(kd == 0), stop=(kd == DT - 1))
                    nc.vector.tensor_copy(out=qp[:, off:off + wd], in_=ps)
                kp = pr_pool.tile([P, KH * P], BF16, tag=f"kp{si}", name=f"kp{si}")
                for (off, wd) in _cuts(KH * P):
                    ps = ps_tile(wd)
                    for kd in range(DT):
                        nc.tensor.matmul(
                            ps, lhsT=w_sb[:, kd, P:2 * P],
                            rhs=xk[:, kd, off:off + wd],
                            start=(kd == 0), stop=(kd == DT - 1))
                    nc.vector.tensor_copy(out=kp[:, off:off + wd], in_=ps)
                vp = pr_pool.tile([P, KH, 2, E], BF16, tag=f"vp{si}", name=f"vp{si}")
                for kt in range(KH):
                    ps = ps_tile(2 * DK)
                    for kd in range(DT):
                        nc.tensor.matmul(
                            ps, lhsT=xv[:, kd, kt * P:(kt + 1) * P],
                            rhs=w_sb[:, kd, 2 * P:3 * P],
                            start=(kd == 0), stop=(kd == DT - 1))
                    nc.vector.tensor_copy(
                        out=vp[:, kt, :, :DK],
                        in_=ps.rearrange("p (h d) -> p h d", d=DK))
                nc.vector.tensor_copy(
                    out=vp[:, :, :, DK], in_=om_sb[:, ktbase:ktbase + KH, :])
                pr[si] = (qp, kp, vp)

            def emit_attn(si):
                KH, nq = slots[si]
                qp, kp, vp = pr[si]
                for (qw0, nqw) in _cuts(nq):  # q-windows <= 512
                    avps = [psO.tile([E, 512], F32, tag="av", name="av")
                            for _ in range(2)]
                    # kt blocks at 512-aligned offsets inside a 2-bank tile
                    # (stride `pad`) so no matmul write crosses a PSUM bank.
                    pad = nqw if 512 % nqw == 0 else 512
                    ck = max(1, min(KH, 1024 // pad))
                    kt0 = 0
                    while kt0 < KH:
                        nk_c = min(ck, KH - kt0)
                        pss = [ps_tile(1024).rearrange(
                            "p (c n) -> p c n", n=pad)[:, :nk_c, :nqw]
                            for _ in range(2)]
                        for ktl in range(nk_c):
                            kt = kt0 + ktl
                            for h in range(2):
                                nc.tensor.matmul(
                                    pss[h][:, ktl],
                                    lhsT=kp[DK * h:DK * (h + 1),
                                            kt * P:(kt + 1) * P],
                                    rhs=qp[DK * h:DK * (h + 1), qw0:qw0 + nqw],
                                    start=True, stop=True)
                        a_sb = [None, None]
                        for h in range(2):
                            a_sb[h] = a_pool.tile(
                                [P, 1024], BF16, tag=f"a{h}",
                                name="a").rearrange(
                                    "p (c n) -> p c n", n=pad)[:, :nk_c, :nqw]
                            nc.scalar.activation(
                                out=a_sb[h], in_=pss[h],
                                func=mybir.ActivationFunctionType.Exp,
                                scale=0.125)
                        for ktl in range(nk_c):
                            kt = kt0 + ktl
                            for h in range(2):
                                nc.tensor.matmul(
                                    avps[h][:, :nqw],
                                    lhsT=vp[:, kt, h, :],
                                    rhs=a_sb[h][:, ktl],
                                    start=(kt == 0), stop=(kt == KH - 1))
                        kt0 += nk_c
                    for h in range(2):
                        o_sb = o_pool.tile([E, 512], F32, tag="o", name="o")
                        nc.vector.tensor_copy(
                            out=o_sb[:, :nqw], in_=avps[h][:, :nqw])
                        nc.sync.dma_start(
                            out=out[(si * 2 + h) * E:(si * 2 + h + 1) * E,
                                    qw0:qw0 + nqw],
                            in_=o_sb[:, :nqw])

            # software-pipelined emission: proj(s+1) interleaves ahead of attn(s)
            ktbases = []
            kb = 0
            for (KH, _) in slots:
                ktbases.append(kb)
                kb += KH
            n = len(slots)
            emit_proj(0, ktbases[0])
            for si in range(n):
                if si + 1 < n:
                    emit_proj(si + 1, ktbases[si + 1])
                emit_attn(si)
    nc.finalize()
    return nc


class _Runner:
    """Compile the Bass graph once and expose run() over 8 cores."""

    def __init__(self, key, n_cores=8):
        import jax
        from jax.experimental.shard_map import shard_map
        from jax.sharding import Mesh, PartitionSpec

        self.jax = jax
        self.n_cores = n_cores
        nc = build_bass(key)
        self.nc = nc
        bass2jax.install_neuronx_cc_hook()
        assert nc.dbg_addr is None
        partition_name = (
            nc.partition_id_tensor.name if nc.partition_id_tensor else None
        )

        in_names, out_names, out_avals, zero_outs = [], [], [], []
        for alloc in nc.m.functions[0].allocations:
            if not isinstance(alloc, mybir.MemoryLocationSet):
                continue
            name = alloc.memorylocations[0].name
            if alloc.kind == "ExternalInput":
                if name != partition_name:
                    in_names.append(name)
            elif alloc.kind == "ExternalOutput":
                shape = tuple(alloc.tensor_shape)
                dtype = mybir.dt.np(alloc.dtype)
                out_names.append(name)
                out_avals.append(jax.core.ShapedArray(shape, dtype))
                zero_outs.append(np.zeros(shape, dtype))
        self.in_names = list(in_names)
        self.out_names = out_names
        self.zero_outs = zero_outs
        n_params = len(in_names)
        all_names = in_names + out_names
        if partition_name is not None:
            all_names = all_names + [partition_name]

        def _body(*args):
            operands = list(args)
            if partition_name is not None:
                operands.append(bass2jax.partition_id_tensor())
            outs = bass2jax._bass_exec_p.bind(
                *operands,
                out_avals=tuple(out_avals),
                in_names=tuple(all_names),
                out_names=tuple(out_names),
                lowering_input_output_aliases=(),
                sim_require_finite=True,
                sim_require_nnan=True,
                nc=nc,
            )
            return tuple(outs)

        devices = jax.devices()[:n_cores]
        self.mesh = Mesh(np.asarray(devices), ("core",))
        n_outs = len(out_names)
        in_specs = (PartitionSpec("core"),) * (n_params + n_outs)
        out_specs = (PartitionSpec("core"),) * n_outs
        donate = tuple(range(n_params, n_params + n_outs))
        mapped = shard_map(
            _body, mesh=self.mesh, in_specs=in_specs, out_specs=out_specs,
            check_rep=False,
        )
        self._run_jit = jax.jit(mapped, donate_argnums=donate, keep_unused=True)

    def _concat_inputs(self, in_maps):
        per_core = [[np.asarray(m[n]) for n in self.in_names] for m in in_maps]
        concat = [
            np.concatenate([per_core[c][i] for c in range(self.n_cores)], axis=0)
            for i in range(len(self.in_names))
        ]
        concat += [
            np.concatenate([z] * self.n_cores, axis=0) for z in self.zero_outs
        ]
        return concat

    def run(self, in_maps):
        concat = self._concat_inputs(in_maps)
        outs = self._run_jit(*concat)
        results = [{} for _ in range(self.n_cores)]
        for name, arr in zip(self.out_names, outs):
            arr = np.asarray(arr)
            per = np.split(arr, self.n_cores, axis=0)
            for c in range(self.n_cores):
                results[c][name] = per[c]
        return results


def _get_compiled(key, n_cores=8):
    ck = (key, n_cores)
    if ck not in _COMPILE_CACHE:
        _COMPILE_CACHE[ck] = _Runner(key, n_cores=n_cores)
    return _COMPILE_CACHE[ck]


def _plan(V_len, Q_len):
    """Per-batch exact shapes; slots ordered by ascending work."""
    nkt = np.minimum(S // P, (V_len + P - 1) // P).astype(np.int64)
    nq = np.minimum(S, ((Q_len + 63) // 64) * 64).astype(np.int64)
    work = nkt * nq
    # Ascending work: small slots' compute fills the DMA shadow of the
    # later, larger slots' input transfers.
    order = [int(b) for b in np.argsort(work, kind="stable")]
    slots = [(int((nkt[b] + 1) // 2), int(nq[b])) for b in order]
    return tuple(slots), order


def _fold(arr):
    """[512, n] f32/bf16 -> [128, 4*n] matching sbuf [p, t, n] layout."""
    n = arr.shape[1]
    return np.ascontiguousarray(
        arr.reshape(DT, P, n).transpose(1, 0, 2).reshape(P, DT * n))


def _prep_in_maps(q, k, v, Wq, Wk, Wv, V_len, key, order):
    bf = ml_dtypes.bfloat16
    slots = list(key)
    # shared per-batch prep
    qF, kT, vT, omc = {}, {}, {}, {}
    for si, b in enumerate(order):
        KH, nq = slots[si]
        vl = int(V_len[b])
        qF[b] = _fold(np.ascontiguousarray(q[b].T[:, :nq]).astype(bf))
        kk = np.ascontiguousarray(k[b].T).astype(bf)
        vv = np.ascontiguousarray(v[b].T).astype(bf)
        kk[:, vl:] = 0
        vv[:, vl:] = 0
        kT[b], vT[b] = kk, vv
    in_maps = []
    for c in range(8):
        p, kh = c % 4, c // 4
        cols = slice(p * 2 * DK, (p + 1) * 2 * DK)
        wcat = np.concatenate(
            [Wq[:, cols], Wk[:, cols], Wv[:, cols]], axis=1).astype(bf)
        qparts, kvparts, oparts = [], [], []
        for si, b in enumerate(order):
            KH, nq = slots[si]
            nk = KH * P
            k0 = kh * nk
            ksl = np.zeros((D, nk), bf)
            vsl = np.zeros((D, nk), bf)
            avail = max(0, min(S, k0 + nk) - k0)
            if avail:
                ksl[:, :avail] = kT[b][:, k0:k0 + avail]
                vsl[:, :avail] = vT[b][:, k0:k0 + avail]
            qparts.append(qF[b])
            kvparts.append(_fold(ksl))
            kvparts.append(_fold(vsl))
            gk = k0 + np.arange(nk)  # global k index per (kt, lane)
            msk = (gk < int(V_len[b])).astype(np.float32).reshape(KH, P).T
            oparts.append(np.repeat(msk[:, :, None], 2, axis=2).reshape(P, 2 * KH))
        in_maps.append({
            "qc": np.ascontiguousarray(np.concatenate(qparts, axis=1)),
            "kvc": np.ascontiguousarray(np.concatenate(kvparts, axis=1)),
            "wc": _fold(wcat),
            "om": np.ascontiguousarray(np.concatenate(oparts, axis=1)),
        })
    return in_maps


def _postprocess(results, Q_len, key, order):
    slots = list(key)
    O = np.zeros((B, S, HEADS * DK), dtype=np.float32)
    acc = np.zeros((4, len(slots), 2, E, max(nq for _, nq in slots)),
                   dtype=np.float32)
    for c in range(8):
        r = np.asarray(results[c]["out"], dtype=np.float32)
        p, kh = c % 4, c // 4
        for si in range(len(slots)):
            for j in range(2):
                blk = r[(si * 2 + j) * E:(si * 2 + j + 1) * E, :]
                acc[p, si, j, :, :blk.shape[1]] += blk
    for si, b in enumerate(order):
        KH, nq = slots[si]
        ql = min(int(Q_len[b]), nq)
        for p in range(4):
            for j in range(2):
                head = 2 * p + j
                m = acc[p, si, j]
                o = m[:DK, :ql] / m[DK:DK + 1, :ql]
                O[b, :ql, head * DK:(head + 1) * DK] = o.T
    return O


def _run(q, k, v, Wq, Wk, Wv, V_len, Q_len, bench=False):
    V_len = np.asarray(V_len).astype(np.int64)
    Q_len = np.asarray(Q_len).astype(np.int64)
    key, order = _plan(V_len, Q_len)
    runner = _get_compiled(key)
    in_maps = _prep_in_maps(q, k, v, Wq, Wk, Wv, V_len, key, order)
    results = runner.run(in_maps)
    out = _postprocess(results, Q_len, key, order)
    exec_ns = _bench_hw(runner, in_maps) if bench else None
    return out, exec_ns


def _bench_hw(runner, in_maps):
    """NTFF-profiled execution via run_bass_kernel_spmd(trace=True)."""
    import sys
    import types
    import os
    import shutil
    try:
        import trn_agent_boot.trn_boot as tb
        hook = tb._ntff_profile_via_ctypes('/opt/axon/libaxon_pjrt.so')
        if hook is None:
            return None
        if 'antenv.axon_hooks' not in sys.modules:
            m = types.ModuleType('antenv.axon_hooks')
            m.get_axon_ntff_profile_hook = lambda: hook
            sys.modules['antenv.axon_hooks'] = m
        from concourse import bass_utils
        bass_utils.upload_artifacts = lambda tmpdir: "local://" + tmpdir
        best = None
        for it in range(3):
            tmpdir = "/tmp/ntff_profile_bench"
            shutil.rmtree(tmpdir, ignore_errors=True)
            os.makedirs(tmpdir, exist_ok=True)
            res = bass_utils.run_bass_kernel_spmd(
                runner.nc, in_maps, core_ids=list(range(8)), trace=True,
                trace_cores=[0], tmpdir=tmpdir)
            t = res.exec_time_ns
            print(f"bench iter {it}: {t} ns")
            if t is not None and (best is None or t < best):
                best = t
        return best
    except Exception as e:
        print("bench failed:", e)
        return None


def kernel(q, k, v, Wq, Wk, Wv, V_len, Q_len):
    q = np.asarray(q, dtype=np.float32)
    k = np.asarray(k, dtype=np.float32)
    v = np.asarray(v, dtype=np.float32)
    Wq = np.asarray(Wq, dtype=np.float32)
    Wk = np.asarray(Wk, dtype=np.float32)
    Wv = np.asarray(Wv, dtype=np.float32)
    out, _ = _run(q, k, v, Wq, Wk, Wv, V_len, Q_len, bench=False)
    return out


# revision 45
# speedup vs baseline: 1.0569x; 1.0569x over previous
"""Trainium2 Bass kernel for masked multi-head attention (B=4, S=2048, D=512, H=8, dk=64).

Sharding: every batch is split into 8 uniform slices = (head-pair p, k-half kh);
core c = (p = c%4, kh = c//4) runs FOUR slots (one per batch), each with that
batch's EXACT (ceil(nkt/2) k-tiles, nq) shape.  All cores run an identical
instruction stream (SPMD) but per-core work is near-perfectly balanced because
every core owns exactly 1/8 of every batch's score columns.

Per-core kernel tricks:
  - mask-free softmax: the host zeroes k/v columns >= V_len (and k-tile pads),
    so masked scores are exactly 0 -> exp=1, and the matching vp rows AND the
    appended ones-column (host-supplied mask, not memset) are 0, so masked
    positions contribute 0 to both numerator and denominator.  No exp bias.
  - scores computed transposed (S^T[k, q]) feeding AV directly; the two heads
    of a pair run as concurrent K=64 matmuls in PE row-groups 0-1 / 2-3
    (tile_position auto-derived from kp row offsets 0 / 64).
  - V gets a masked ones-column so the AV matmul also emits denominators.
  - k-half partial sums (numerator rows + denominator row) are combined on the
    host before the divide; host also applies the q mask.
  - inputs are host-prefolded to the exact SBUF layout [128, 4*n] so every
    input DMA is 128 fully-contiguous descriptors (HWDGE via the sync queue);
    outputs drain per q-window on the same ring after inputs complete.
"""

import numpy as np
import ml_dtypes

import concourse.bass as bass
import concourse.tile as tile
from concourse import bacc
from concourse import mybir
from concourse import bass2jax

B, S, D = 4, 2048, 512
HEADS, DK = 8, 64
P = 128
DT = D // P  # 4 contraction chunks over D
E = DK + 1   # 64 output dims + denominator row

BF16 = mybir.dt.bfloat16
F32 = mybir.dt.float32

_COMPILE_CACHE = {}


def _cuts(n, w=512):
    return [(s, min(w, n - s)) for s in range(0, n, w)]


def build_bass(key):
    """key: tuple of (KH, nq) per slot, compute order (ascending work)."""
    nc = bacc.Bacc(None, target_bir_lowering=False, debug=False)
    slots = list(key)
    KTS = sum(kh for kh, _ in slots)
    NQS = sum(nq for _, nq in slots)
    NKS = sum(kh * P for kh, _ in slots)
    max_nq = max(nq for _, nq in slots)

    qc = nc.declare_dram_parameter("qc", [P, DT * NQS], BF16, isOutput=False)
    kvc = nc.declare_dram_parameter(
        "kvc", [P, 2 * DT * NKS], BF16, isOutput=False)
    wc = nc.declare_dram_parameter("wc", [P, DT * 3 * P], BF16, isOutput=False)
    om = nc.declare_dram_parameter("om", [P, KTS * 2], F32, isOutput=False)
    out = nc.declare_dram_parameter(
        "out", [len(slots) * 2 * E, max_nq], F32, isOutput=True)

    with tile.TileContext(nc) as tc:
        with (
            tc.tile_pool(name="w", bufs=1) as w_pool,
            tc.tile_pool(name="x", bufs=1) as x_pool,
            tc.tile_pool(name="pr", bufs=1) as pr_pool,
            tc.tile_pool(name="aexp", bufs=3) as a_pool,
            tc.tile_pool(name="osb", bufs=2) as o_pool,
            tc.tile_pool(name="psS", bufs=3, space="PSUM") as psS,
            tc.tile_pool(name="psO", bufs=2, space="PSUM") as psO,
        ):
            w_sb = w_pool.tile([P, DT, 3 * P], BF16, name="w")
            nc.sync.dma_start(
                out=w_sb, in_=wc.rearrange("p (t n) -> p t n", t=DT))
            om_sb = w_pool.tile([P, KTS, 2], F32, name="om")
            nc.gpsimd.dma_start(
                out=om_sb, in_=om.rearrange("p (t n) -> p t n", n=2))

            x_sb = {}
            qoff = koff = 0
            for si, (KH, nq) in enumerate(slots):
                nk = KH * P
                xq = x_pool.tile([P, DT, nq], BF16, tag=f"xq{si}", name=f"xq{si}")
                nc.gpsimd.dma_start(
                    out=xq,
                    in_=qc[:, DT * qoff:DT * (qoff + nq)].rearrange(
                        "p (t n) -> p t n", t=DT))
                xkv = x_pool.tile([P, 2, DT, nk], BF16, tag=f"xkv{si}",
                                  name=f"xkv{si}")
                kvblk = kvc[:, 2 * DT * koff:2 * DT * (koff + nk)].rearrange(
                    "p (u t n) -> p u t n", u=2, t=DT)
                nc.sync.dma_start(out=xkv[:, 0], in_=kvblk[:, 0])
                nc.sync.dma_start(out=xkv[:, 1], in_=kvblk[:, 1])
                x_sb[si] = (xq, xkv[:, 0], xkv[:, 1])
                qoff += nq
                koff += nk

            def ps_tile(width):
                t = psS.tile([P, 1024], F32, tag="s", name="ps")
                return t[:, :width]

            pr = {}

            def emit_proj(si, ktbase):
                KH, nq = slots[si]
                xq, xk, xv = x_sb[si]
                qp = pr_pool.tile([P, nq], BF16, tag=f"qp{si}", name=f"qp{si}")
                for (off, wd) in _cuts(nq):
                    ps = ps_tile(wd)
                    for kd in range(DT):
                        nc.tensor.matmul(
                            ps, lhsT=w_sb[:, kd, 0:P],
                            rhs=xq[:, kd, off:off + wd],
                            start=(kd == 0), stop=(kd == DT - 1))
                    nc.vector.tensor_copy(out=qp[:, off:off + wd], in_=ps)
                kp = pr_pool.tile([P, KH * P], BF16, tag=f"kp{si}", name=f"kp{si}")
                for (off, wd) in _cuts(KH * P):
                    ps = ps_tile(wd)
                    for kd in range(DT):
                        nc.tensor.matmul(
                            ps, lhsT=w_sb[:, kd, P:2 * P],
                            rhs=xk[:, kd, off:off + wd],
                            start=(kd == 0), stop=(kd == DT - 1))
                    nc.vector.tensor_copy(out=kp[:, off:off + wd], in_=ps)
                vp = pr_pool.tile([P, KH, 2, E], BF16, tag=f"vp{si}", name=f"vp{si}")
                for kt in range(KH):
                    ps = ps_tile(2 * DK)
                    for kd in range(DT):
                        nc.tensor.matmul(
                            ps, lhsT=xv[:, kd, kt * P:(kt + 1) * P],
                            rhs=w_sb[:, kd, 2 * P:3 * P],
                            start=(kd == 0), stop=(kd == DT - 1))
                    nc.vector.tensor_copy(
                        out=vp[:, kt, :, :DK],
                        in_=ps.rearrange("p (h d) -> p h d", d=DK))
                nc.vector.tensor_copy(
                    out=vp[:, :, :, DK], in_=om_sb[:, ktbase:ktbase + KH, :])
                pr[si] = (qp, kp, vp)

            def emit_attn(si):
                KH, nq = slots[si]
                qp, kp, vp = pr[si]
                for (qw0, nqw) in _cuts(nq):  # q-windows <= 512
                    avps = [psO.tile([E, 512], F32, tag="av", name="av")
                            for _ in range(2)]
                    # kt blocks at 512-aligned offsets inside a 2-bank tile
                    # (stride `pad`) so no matmul write crosses a PSUM bank.
                    pad = nqw if 512 % nqw == 0 else 512
                    ck = max(1, min(KH, 1024 // pad))
                    kt0 = 0
                    while kt0 < KH:
                        nk_c = min(ck, KH - kt0)
                        pss = [ps_tile(1024).rearrange(
                            "p (c n) -> p c n", n=pad)[:, :nk_c, :nqw]
                            for _ in range(2)]
                        for ktl in range(nk_c):
                            kt = kt0 + ktl
                            for h in range(2):
                                nc.tensor.matmul(
                                    pss[h][:, ktl],
                                    lhsT=kp[DK * h:DK * (h + 1),
                                            kt * P:(kt + 1) * P],
                                    rhs=qp[DK * h:DK * (h + 1), qw0:qw0 + nqw],
                                    start=True, stop=True)
                        a_sb = [None, None]
                        for h in range(2):
                            a_sb[h] = a_pool.tile(
                                [P, 1024], BF16, tag=f"a{h}",
                                name="a").rearrange(
                                    "p (c n) -> p c n", n=pad)[:, :nk_c, :nqw]
                            nc.scalar.activation(
                                out=a_sb[h], in_=pss[h],
                                func=mybir.ActivationFunctionType.Exp,
                                scale=0.125)
                        for ktl in range(nk_c):
                            kt = kt0 + ktl
                            for h in range(2):
                                nc.tensor.matmul(
                                    avps[h][:, :nqw],
                                    lhsT=vp[:, kt, h, :],
                                    rhs=a_sb[h][:, ktl],
                                    start=(kt == 0), stop=(kt == KH - 1))
                        kt0 += nk_c
                    for h in range(2):
                        o_sb = o_pool.tile([E, 512], F32, tag="o", name="o")
                        nc.vector.tensor_copy(
                            out=o_sb[:, :nqw], in_=avps[h][:, :nqw])
                        nc.sync.dma_start(
                            out=out[(si * 2 + h) * E:(si * 2 + h + 1) * E,
                                    qw0:qw0 + nqw],
                            in_=o_sb[:, :nqw])

            # software-pipelined emission: proj(s+1) interleaves ahead of attn(s)
            ktbases = []
            kb = 0
            for (KH, _) in slots:
                ktbases.append(kb)
                kb += KH
            n = len(slots)
            emit_proj(0, ktbases[0])
            for si in range(n):
                if si + 1 < n:
                    emit_proj(si + 1, ktbases[si + 1])
                emit_attn(si)
    nc.finalize()
    return nc


class _Runner:
    """Compile the Bass graph once and expose run() over 8 cores."""

    def __init__(self, key, n_cores=8):
        import jax
        from jax.experimental.shard_map import shard_map
        from jax.sharding import Mesh, PartitionSpec

        self.jax = jax
        self.n_cores = n_cores
        nc = build_bass(key)
        self.nc = nc
        bass2jax.install_neuronx_cc_hook()
        assert nc.dbg_addr is None
        partition_name = (
            nc.partition_id_tensor.name if nc.partition_id_tensor else None
        )

        in_names, out_names, out_avals, zero_outs = [], [], [], []
        for alloc in nc.m.functions[0].allocations:
            if not isinstance(alloc, mybir.MemoryLocationSet):
                continue
            name = alloc.memorylocations[0].name
            if alloc.kind == "ExternalInput":
                if name != partition_name:
                    in_names.append(name)
            elif alloc.kind == "ExternalOutput":
                shape = tuple(alloc.tensor_shape)
                dtype = mybir.dt.np(alloc.dtype)
                out_names.append(name)
                out_avals.append(jax.core.ShapedArray(shape, dtype))
                zero_outs.append(np.zeros(shape, dtype))
        self.in_names = list(in_names)
        self.out_names = out_names
        self.zero_outs = zero_outs
        n_params = len(in_names)
        all_names = in_names + out_names
        if partition_name is not None:
            all_names = all_names + [partition_name]

        def _body(*args):
            operands = list(args)
            if partition_name is not None:
                operands.append(bass2jax.partition_id_tensor())
            outs = bass2jax._bass_exec_p.bind(
                *operands,
                out_avals=tuple(out_avals),
                in_names=tuple(all_names),
                out_names=tuple(out_names),
                lowering_input_output_aliases=(),
                sim_require_finite=True,
                sim_require_nnan=True,
                nc=nc,
            )
            return tuple(outs)

        devices = jax.devices()[:n_cores]
        self.mesh = Mesh(np.asarray(devices), ("core",))
        n_outs = len(out_names)
        in_specs = (PartitionSpec("core"),) * (n_params + n_outs)
        out_specs = (PartitionSpec("core"),) * n_outs
        donate = tuple(range(n_params, n_params + n_outs))
        mapped = shard_map(
            _body, mesh=self.mesh, in_specs=in_specs, out_specs=out_specs,
            check_rep=False,
        )
        self._run_jit = jax.jit(mapped, donate_argnums=donate, keep_unused=True)

    def _concat_inputs(self, in_maps):
        per_core = [[np.asarray(m[n]) for n in self.in_names] for m in in_maps]
        concat = [
            np.concatenate([per_core[c][i] for c in range(self.n_cores)], axis=0)
            for i in range(len(self.in_names))
        ]
        concat += [
            np.concatenate([z] * self.n_cores, axis=0) for z in self.zero_outs
        ]
        return concat

    def run(self, in_maps):
        concat = self._concat_inputs(in_maps)
        outs = self._run_jit(*concat)
        results = [{} for _ in range(self.n_cores)]
        for name, arr in zip(self.out_names, outs):
            arr = np.asarray(arr)
            per = np.split(arr, self.n_cores, axis=0)
            for c in range(self.n_cores):
                results[c][name] = per[c]
        return results


def _get_compiled(key, n_cores=8):
    ck = (key, n_cores)
    if ck not in _COMPILE_CACHE:
        _COMPILE_CACHE[ck] = _Runner(key, n_cores=n_cores)
    return _COMPILE_CACHE[ck]


def _plan(V_len, Q_len):
    """Per-batch exact shapes; slots ordered by ascending work."""
    nkt = np.minimum(S // P, (V_len + P - 1) // P).astype(np.int64)
    nq = np.minimum(S, ((Q_len + 63) // 64) * 64).astype(np.int64)
    work = nkt * nq
    # Ascending work: small slots' compute fills the DMA shadow of the
    # later, larger slots' input transfers.
    order = [int(b) for b in np.argsort(work, kind="stable")]
    slots = [(int((nkt[b] + 1) // 2), int(nq[b])) for b in order]
    return tuple(slots), order


def _fold(arr):
    """[512, n] f32/bf16 -> [128, 4*n] matching sbuf [p, t, n] layout."""
    n = arr.shape[1]
    return np.ascontiguousarray(
        arr.reshape(DT, P, n).transpose(1, 0, 2).reshape(P, DT * n))


def _prep_in_maps(q, k, v, Wq, Wk, Wv, V_len, key, order):
    bf = ml_dtypes.bfloat16
    slots = list(key)
    # shared per-batch prep
    qF, kT, vT, omc = {}, {}, {}, {}
    for si, b in enumerate(order):
        KH, nq = slots[si]
        vl = int(V_len[b])
        qF[b] = _fold(np.ascontiguousarray(q[b].T[:, :nq]).astype(bf))
        kk = np.ascontiguousarray(k[b].T).astype(bf)
        vv = np.ascontiguousarray(v[b].T).astype(bf)
        kk[:, vl:] = 0
        vv[:, vl:] = 0
        kT[b], vT[b] = kk, vv
    in_maps = []
    for c in range(8):
        p, kh = c % 4, c // 4
        cols = slice(p * 2 * DK, (p + 1) * 2 * DK)
        wcat = np.concatenate(
            [Wq[:, cols], Wk[:, cols], Wv[:, cols]], axis=1).astype(bf)
        qparts, kvparts, oparts = [], [], []
        for si, b in enumerate(order):
            KH, nq = slots[si]
            nk = KH * P
            k0 = kh * nk
            ksl = np.zeros((D, nk), bf)
            vsl = np.zeros((D, nk), bf)
            avail = max(0, min(S, k0 + nk) - k0)
            if avail:
                ksl[:, :avail] = kT[b][:, k0:k0 + avail]
                vsl[:, :avail] = vT[b][:, k0:k0 + avail]
            qparts.append(qF[b])
            kvparts.append(_fold(ksl))
            kvparts.append(_fold(vsl))
            gk = k0 + np.arange(nk)  # global k index per (kt, lane)
            msk = (gk < int(V_len[b])).astype(np.float32).reshape(KH, P).T
            oparts.append(np.repeat(msk[:, :, None], 2, axis=2).reshape(P, 2 * KH))
        in_maps.append({
            "qc": np.ascontiguousarray(np.concatenate(qparts, axis=1)),
            "kvc": np.ascontiguousarray(np.concatenate(kvparts, axis=1)),
            "wc": _fold(wcat),
            "om": np.ascontiguousarray(np.concatenate(oparts, axis=1)),
        })
    return in_maps


def _postprocess(results, Q_len, key, order):
    slots = list(key)
    O = np.zeros((B, S, HEADS * DK), dtype=np.float32)
    acc = np.zeros((4, len(slots), 2, E, max(nq for _, nq in slots)),
                   dtype=np.float32)
    for c in range(8):
        r = np.asarray(results[c]["out"], dtype=np.float32)
        p, kh = c % 4, c // 4
        for si in range(len(slots)):
            for j in range(2):
                blk = r[(si * 2 + j) * E:(si * 2 + j + 1) * E, :]
                acc[p, si, j, :, :blk.shape[1]] += blk
    for si, b in enumerate(order):
        KH, nq = slots[si]
        ql = min(int(Q_len[b]), nq)
        for p in range(4):
            for j in range(2):
                head = 2 * p + j
                m = acc[p, si, j]
                o = m[:DK, :ql] / m[DK:DK + 1, :ql]
                O[b, :ql, head * DK:(head + 1) * DK] = o.T
    return O


def _run(q, k, v, Wq, Wk, Wv, V_len, Q_len, bench=False):
    V_len = np.asarray(V_len).astype(np.int64)
    Q_len = np.asarray(Q_len).astype(np.int64)
    key, order = _plan(V_len, Q_len)
    runner = _get_compiled(key)
    in_maps = _prep_in_maps(q, k, v, Wq, Wk, Wv, V_len, key, order)
    results = runner.run(in_maps)
    out = _postprocess(results, Q_len, key, order)
    exec_ns = _bench_hw(runner, in_maps) if bench else None
    return out, exec_ns


def _bench_hw(runner, in_maps):
    """NTFF-profiled execution via run_bass_kernel_spmd(trace=True)."""
    import sys
    import types
    import os
    import shutil
    try:
        import trn_agent_boot.trn_boot as tb
        hook = tb._ntff_profile_via_ctypes('/opt/axon/libaxon_pjrt.so')
        if hook is None:
            return None
        if 'antenv.axon_hooks' not in sys.modules:
            m = types.ModuleType('antenv.axon_hooks')
            m.get_axon_ntff_profile_hook = lambda: hook
            sys.modules['antenv.axon_hooks'] = m
        from concourse import bass_utils
        bass_utils.upload_artifacts = lambda tmpdir: "local://" + tmpdir
        best = None
        for it in range(3):
            tmpdir = "/tmp/ntff_profile_bench"
            shutil.rmtree(tmpdir, ignore_errors=True)
            os.makedirs(tmpdir, exist_ok=True)
            res = bass_utils.run_bass_kernel_spmd(
                runner.nc, in_maps, core_ids=list(range(8)), trace=True,
                trace_cores=[0], tmpdir=tmpdir)
            t = res.exec_time_ns
            print(f"bench iter {it}: {t} ns")
            if t is not None and (best is None or t < best):
                best = t
        return best
    except Exception as e:
        print("bench failed:", e)
        return None


def kernel(q, k, v, Wq, Wk, Wv, V_len, Q_len):
    q = np.asarray(q, dtype=np.float32)
    k = np.asarray(k, dtype=np.float32)
    v = np.asarray(v, dtype=np.float32)
    Wq = np.asarray(Wq, dtype=np.float32)
    Wk = np.asarray(Wk, dtype=np.float32)
    Wv = np.asarray(Wv, dtype=np.float32)
    out, _ = _run(q, k, v, Wq, Wk, Wv, V_len, Q_len, bench=False)
    return out


# revision 46
# speedup vs baseline: 1.1322x; 1.0712x over previous
"""Trainium2 Bass kernel for masked multi-head attention (B=4, S=2048, D=512, H=8, dk=64).

Sharding: every batch is split into 8 uniform slices = (head-pair p, k-half kh);
core c = (p = c%4, kh = c//4) runs FOUR slots (one per batch), each with that
batch's EXACT (ceil(nkt/2) k-tiles, nq) shape.  All cores run an identical
instruction stream (SPMD) but per-core work is near-perfectly balanced because
every core owns exactly 1/8 of every batch's score columns.

Per-core kernel tricks:
  - mask-free softmax: the host zeroes k/v columns >= V_len (and k-tile pads),
    so masked scores are exactly 0 -> exp=1, and the matching vp rows AND the
    appended ones-column (host-supplied mask, not memset) are 0, so masked
    positions contribute 0 to both numerator and denominator.  No exp bias.
  - scores computed transposed (S^T[k, q]) feeding AV directly; the two heads
    of a pair run as concurrent K=64 matmuls in PE row-groups 0-1 / 2-3
    (tile_position auto-derived from kp row offsets 0 / 64).
  - V gets a masked ones-column so the AV matmul also emits denominators.
  - k-half partial sums (numerator rows + denominator row) are combined on the
    host before the divide; host also applies the q mask.
  - inputs are host-prefolded to the exact SBUF layout [128, 4*n] so every
    input DMA is 128 fully-contiguous descriptors (HWDGE via the sync queue);
    outputs drain per q-window on the same ring after inputs complete.
"""

import numpy as np
import ml_dtypes

import concourse.bass as bass
import concourse.tile as tile
from concourse import bacc
from concourse import mybir
from concourse import bass2jax

B, S, D = 4, 2048, 512
HEADS, DK = 8, 64
P = 128
DT = D // P  # 4 contraction chunks over D
E = DK + 1   # 64 output dims + denominator row

BF16 = mybir.dt.bfloat16
F32 = mybir.dt.float32

_COMPILE_CACHE = {}


def _cuts(n, w=512):
    return [(s, min(w, n - s)) for s in range(0, n, w)]


def build_bass(key):
    """key: tuple of (KH, nq) per slot, compute order (ascending work)."""
    nc = bacc.Bacc(None, target_bir_lowering=False, debug=False)
    slots = list(key)
    KTS = sum(kh for kh, _ in slots)
    NQS = sum(nq for _, nq in slots)
    NKS = sum(kh * P for kh, _ in slots)
    max_nq = max(nq for _, nq in slots)

    qc = nc.declare_dram_parameter("qc", [P, DT * NQS], BF16, isOutput=False)
    kvc = nc.declare_dram_parameter(
        "kvc", [P, 2 * DT * NKS], BF16, isOutput=False)
    wc = nc.declare_dram_parameter("wc", [P, DT * 3 * P], BF16, isOutput=False)
    om = nc.declare_dram_parameter("om", [P, KTS * 2], F32, isOutput=False)
    out = nc.declare_dram_parameter(
        "out", [len(slots) * 2 * E, max_nq], F32, isOutput=True)

    with tile.TileContext(nc) as tc:
        with (
            tc.tile_pool(name="w", bufs=1) as w_pool,
            tc.tile_pool(name="x", bufs=1) as x_pool,
            tc.tile_pool(name="pr", bufs=1) as pr_pool,
            tc.tile_pool(name="aexp", bufs=3) as a_pool,
            tc.tile_pool(name="osb", bufs=2) as o_pool,
            tc.tile_pool(name="psS", bufs=3, space="PSUM") as psS,
            tc.tile_pool(name="psO", bufs=2, space="PSUM") as psO,
        ):
            w_sb = w_pool.tile([P, DT, 3 * P], BF16, name="w")
            nc.sync.dma_start(
                out=w_sb, in_=wc.rearrange("p (t n) -> p t n", t=DT))
            om_sb = w_pool.tile([P, KTS, 2], F32, name="om")
            nc.sync.dma_start(
                out=om_sb, in_=om.rearrange("p (t n) -> p t n", n=2))

            x_sb = {}
            qoff = koff = 0
            for si, (KH, nq) in enumerate(slots):
                nk = KH * P
                xq = x_pool.tile([P, DT, nq], BF16, tag=f"xq{si}", name=f"xq{si}")
                nc.sync.dma_start(
                    out=xq,
                    in_=qc[:, DT * qoff:DT * (qoff + nq)].rearrange(
                        "p (t n) -> p t n", t=DT))
                xkv = x_pool.tile([P, 2, DT, nk], BF16, tag=f"xkv{si}",
                                  name=f"xkv{si}")
                kvblk = kvc[:, 2 * DT * koff:2 * DT * (koff + nk)].rearrange(
                    "p (u t n) -> p u t n", u=2, t=DT)
                nc.sync.dma_start(out=xkv[:, 0], in_=kvblk[:, 0])
                nc.sync.dma_start(out=xkv[:, 1], in_=kvblk[:, 1])
                x_sb[si] = (xq, xkv[:, 0], xkv[:, 1])
                qoff += nq
                koff += nk

            def ps_tile(width):
                t = psS.tile([P, 1024], F32, tag="s", name="ps")
                return t[:, :width]

            pr = {}

            def emit_proj(si, ktbase):
                KH, nq = slots[si]
                xq, xk, xv = x_sb[si]
                qp = pr_pool.tile([P, nq], BF16, tag=f"qp{si}", name=f"qp{si}")
                for (off, wd) in _cuts(nq):
                    ps = ps_tile(wd)
                    for kd in range(DT):
                        nc.tensor.matmul(
                            ps, lhsT=w_sb[:, kd, 0:P],
                            rhs=xq[:, kd, off:off + wd],
                            start=(kd == 0), stop=(kd == DT - 1))
                    nc.vector.tensor_copy(out=qp[:, off:off + wd], in_=ps)
                kp = pr_pool.tile([P, KH * P], BF16, tag=f"kp{si}", name=f"kp{si}")
                for (off, wd) in _cuts(KH * P):
                    ps = ps_tile(wd)
                    for kd in range(DT):
                        nc.tensor.matmul(
                            ps, lhsT=w_sb[:, kd, P:2 * P],
                            rhs=xk[:, kd, off:off + wd],
                            start=(kd == 0), stop=(kd == DT - 1))
                    nc.vector.tensor_copy(out=kp[:, off:off + wd], in_=ps)
                vp = pr_pool.tile([P, KH, 2, E], BF16, tag=f"vp{si}", name=f"vp{si}")
                for kt in range(KH):
                    ps = ps_tile(2 * DK)
                    for kd in range(DT):
                        nc.tensor.matmul(
                            ps, lhsT=xv[:, kd, kt * P:(kt + 1) * P],
                            rhs=w_sb[:, kd, 2 * P:3 * P],
                            start=(kd == 0), stop=(kd == DT - 1))
                    nc.vector.tensor_copy(
                        out=vp[:, kt, :, :DK],
                        in_=ps.rearrange("p (h d) -> p h d", d=DK))
                nc.vector.tensor_copy(
                    out=vp[:, :, :, DK], in_=om_sb[:, ktbase:ktbase + KH, :])
                pr[si] = (qp, kp, vp)

            def emit_attn(si):
                KH, nq = slots[si]
                qp, kp, vp = pr[si]
                for (qw0, nqw) in _cuts(nq):  # q-windows <= 512
                    avps = [psO.tile([E, 512], F32, tag="av", name="av")
                            for _ in range(2)]
                    # kt blocks at 512-aligned offsets inside a 2-bank tile
                    # (stride `pad`) so no matmul write crosses a PSUM bank.
                    pad = nqw if 512 % nqw == 0 else 512
                    ck = max(1, min(KH, 1024 // pad))
                    kt0 = 0
                    while kt0 < KH:
                        nk_c = min(ck, KH - kt0)
                        pss = [ps_tile(1024).rearrange(
                            "p (c n) -> p c n", n=pad)[:, :nk_c, :nqw]
                            for _ in range(2)]
                        for ktl in range(nk_c):
                            kt = kt0 + ktl
                            for h in range(2):
                                nc.tensor.matmul(
                                    pss[h][:, ktl],
                                    lhsT=kp[DK * h:DK * (h + 1),
                                            kt * P:(kt + 1) * P],
                                    rhs=qp[DK * h:DK * (h + 1), qw0:qw0 + nqw],
                                    start=True, stop=True)
                        a_sb = [None, None]
                        for h in range(2):
                            a_sb[h] = a_pool.tile(
                                [P, 1024], BF16, tag=f"a{h}",
                                name="a").rearrange(
                                    "p (c n) -> p c n", n=pad)[:, :nk_c, :nqw]
                            nc.scalar.activation(
                                out=a_sb[h], in_=pss[h],
                                func=mybir.ActivationFunctionType.Exp,
                                scale=0.125)
                        for ktl in range(nk_c):
                            kt = kt0 + ktl
                            for h in range(2):
                                nc.tensor.matmul(
                                    avps[h][:, :nqw],
                                    lhsT=vp[:, kt, h, :],
                                    rhs=a_sb[h][:, ktl],
                                    start=(kt == 0), stop=(kt == KH - 1))
                        kt0 += nk_c
                    for h in range(2):
                        o_sb = o_pool.tile([E, 512], F32, tag="o", name="o")
                        nc.vector.tensor_copy(
                            out=o_sb[:, :nqw], in_=avps[h][:, :nqw])
                        nc.sync.dma_start(
                            out=out[(si * 2 + h) * E:(si * 2 + h + 1) * E,
                                    qw0:qw0 + nqw],
                            in_=o_sb[:, :nqw])

            # software-pipelined emission: proj(s+1) interleaves ahead of attn(s)
            ktbases = []
            kb = 0
            for (KH, _) in slots:
                ktbases.append(kb)
                kb += KH
            n = len(slots)
            emit_proj(0, ktbases[0])
            for si in range(n):
                if si + 1 < n:
                    emit_proj(si + 1, ktbases[si + 1])
                emit_attn(si)
    nc.finalize()
    return nc


class _Runner:
    """Compile the Bass graph once and expose run() over 8 cores."""

    def __init__(self, key, n_cores=8):
        import jax
        from jax.experimental.shard_map import shard_map
        from jax.sharding import Mesh, PartitionSpec

        self.jax = jax
        self.n_cores = n_cores
        nc = build_bass(key)
        self.nc = nc
        bass2jax.install_neuronx_cc_hook()
        assert nc.dbg_addr is None
        partition_name = (
            nc.partition_id_tensor.name if nc.partition_id_tensor else None
        )

        in_names, out_names, out_avals, zero_outs = [], [], [], []
        for alloc in nc.m.functions[0].allocations:
            if not isinstance(alloc, mybir.MemoryLocationSet):
                continue
            name = alloc.memorylocations[0].name
            if alloc.kind == "ExternalInput":
                if name != partition_name:
                    in_names.append(name)
            elif alloc.kind == "ExternalOutput":
                shape = tuple(alloc.tensor_shape)
                dtype = mybir.dt.np(alloc.dtype)
                out_names.append(name)
                out_avals.append(jax.core.ShapedArray(shape, dtype))
                zero_outs.append(np.zeros(shape, dtype))
        self.in_names = list(in_names)
        self.out_names = out_names
        self.zero_outs = zero_outs
        n_params = len(in_names)
        all_names = in_names + out_names
        if partition_name is not None:
            all_names = all_names + [partition_name]

        def _body(*args):
            operands = list(args)
            if partition_name is not None:
                operands.append(bass2jax.partition_id_tensor())
            outs = bass2jax._bass_exec_p.bind(
                *operands,
                out_avals=tuple(out_avals),
                in_names=tuple(all_names),
                out_names=tuple(out_names),
                lowering_input_output_aliases=(),
                sim_require_finite=True,
                sim_require_nnan=True,
                nc=nc,
            )
            return tuple(outs)

        devices = jax.devices()[:n_cores]
        self.mesh = Mesh(np.asarray(devices), ("core",))
        n_outs = len(out_names)
        in_specs = (PartitionSpec("core"),) * (n_params + n_outs)
        out_specs = (PartitionSpec("core"),) * n_outs
        donate = tuple(range(n_params, n_params + n_outs))
        mapped = shard_map(
            _body, mesh=self.mesh, in_specs=in_specs, out_specs=out_specs,
            check_rep=False,
        )
        self._run_jit = jax.jit(mapped, donate_argnums=donate, keep_unused=True)

    def _concat_inputs(self, in_maps):
        per_core = [[np.asarray(m[n]) for n in self.in_names] for m in in_maps]
        concat = [
            np.concatenate([per_core[c][i] for c in range(self.n_cores)], axis=0)
            for i in range(len(self.in_names))
        ]
        concat += [
            np.concatenate([z] * self.n_cores, axis=0) for z in self.zero_outs
        ]
        return concat

    def run(self, in_maps):
        concat = self._concat_inputs(in_maps)
        outs = self._run_jit(*concat)
        results = [{} for _ in range(self.n_cores)]
        for name, arr in zip(self.out_names, outs):
            arr = np.asarray(arr)
            per = np.split(arr, self.n_cores, axis=0)
            for c in range(self.n_cores):
                results[c][name] = per[c]
        return results


def _get_compiled(key, n_cores=8):
    ck = (key, n_cores)
    if ck not in _COMPILE_CACHE:
        _COMPILE_CACHE[ck] = _Runner(key, n_cores=n_cores)
    return _COMPILE_CACHE[ck]


def _plan(V_len, Q_len):
    """Per-batch exact shapes; slots ordered by ascending work."""
    nkt = np.minimum(S // P, (V_len + P - 1) // P).astype(np.int64)
    nq = np.minimum(S, ((Q_len + 63) // 64) * 64).astype(np.int64)
    work = nkt * nq
    # Ascending work: small slots' compute fills the DMA shadow of the
    # later, larger slots' input transfers.
    order = [int(b) for b in np.argsort(work, kind="stable")]
    slots = [(int((nkt[b] + 1) // 2), int(nq[b])) for b in order]
    return tuple(slots), order


def _fold(arr):
    """[512, n] f32/bf16 -> [128, 4*n] matching sbuf [p, t, n] layout."""
    n = arr.shape[1]
    return np.ascontiguousarray(
        arr.reshape(DT, P, n).transpose(1, 0, 2).reshape(P, DT * n))


def _prep_in_maps(q, k, v, Wq, Wk, Wv, V_len, key, order):
    bf = ml_dtypes.bfloat16
    slots = list(key)
    # shared per-batch prep
    qF, kT, vT, omc = {}, {}, {}, {}
    for si, b in enumerate(order):
        KH, nq = slots[si]
        vl = int(V_len[b])
        qF[b] = _fold(np.ascontiguousarray(q[b].T[:, :nq]).astype(bf))
        kk = np.ascontiguousarray(k[b].T).astype(bf)
        vv = np.ascontiguousarray(v[b].T).astype(bf)
        kk[:, vl:] = 0
        vv[:, vl:] = 0
        kT[b], vT[b] = kk, vv
    in_maps = []
    for c in range(8):
        p, kh = c % 4, c // 4
        cols = slice(p * 2 * DK, (p + 1) * 2 * DK)
        wcat = np.concatenate(
            [Wq[:, cols], Wk[:, cols], Wv[:, cols]], axis=1).astype(bf)
        qparts, kvparts, oparts = [], [], []
        for si, b in enumerate(order):
            KH, nq = slots[si]
            nk = KH * P
            k0 = kh * nk
            ksl = np.zeros((D, nk), bf)
            vsl = np.zeros((D, nk), bf)
            avail = max(0, min(S, k0 + nk) - k0)
            if avail:
                ksl[:, :avail] = kT[b][:, k0:k0 + avail]
                vsl[:, :avail] = vT[b][:, k0:k0 + avail]
            qparts.append(qF[b])
            kvparts.append(_fold(ksl))
            kvparts.append(_fold(vsl))
            gk = k0 + np.arange(nk)  # global k index per (kt, lane)
            msk = (gk < int(V_len[b])).astype(np.float32).reshape(KH, P).T
            oparts.append(np.repeat(msk[:, :, None], 2, axis=2).reshape(P, 2 * KH))
        in_maps.append({
            "qc": np.ascontiguousarray(np.concatenate(qparts, axis=1)),
            "kvc": np.ascontiguousarray(np.concatenate(kvparts, axis=1)),
            "wc": _fold(wcat),
            "om": np.ascontiguousarray(np.concatenate(oparts, axis=1)),
        })
    return in_maps


def _postprocess(results, Q_len, key, order):
    slots = list(key)
    O = np.zeros((B, S, HEADS * DK), dtype=np.float32)
    acc = np.zeros((4, len(slots), 2, E, max(nq for _, nq in slots)),
                   dtype=np.float32)
    for c in range(8):
        r = np.asarray(results[c]["out"], dtype=np.float32)
        p, kh = c % 4, c // 4
        for si in range(len(slots)):
            for j in range(2):
                blk = r[(si * 2 + j) * E:(si * 2 + j + 1) * E, :]
                acc[p, si, j, :, :blk.shape[1]] += blk
    for si, b in enumerate(order):
        KH, nq = slots[si]
        ql = min(int(Q_len[b]), nq)
        for p in range(4):
            for j in range(2):
                head = 2 * p + j
                m = acc[p, si, j]
                o = m[:DK, :ql] / m[DK:DK + 1, :ql]
                O[b, :ql, head * DK:(head + 1) * DK] = o.T
    return O


def _run(q, k, v, Wq, Wk, Wv, V_len, Q_len, bench=False):
    V_len = np.asarray(V_len).astype(np.int64)
    Q_len = np.asarray(Q_len).astype(np.int64)
    key, order = _plan(V_len, Q_len)
    runner = _get_compiled(key)
    in_maps = _prep_in_maps(q, k, v, Wq, Wk, Wv, V_len, key, order)
    results = runner.run(in_maps)
    out = _postprocess(results, Q_len, key, order)
    exec_ns = _bench_hw(runner, in_maps) if bench else None
    return out, exec_ns


def _bench_hw(runner, in_maps):
    """NTFF-profiled execution via run_bass_kernel_spmd(trace=True)."""
    import sys
    import types
    import os
    import shutil
    try:
        import trn_agent_boot.trn_boot as tb
        hook = tb._ntff_profile_via_ctypes('/opt/axon/libaxon_pjrt.so')
        if hook is None:
            return None
        if 'antenv.axon_hooks' not in sys.modules:
            m = types.ModuleType('antenv.axon_hooks')
            m.get_axon_ntff_profile_hook = lambda: hook
            sys.modules['antenv.axon_hooks'] = m
        from concourse import bass_utils
        bass_utils.upload_artifacts = lambda tmpdir: "local://" + tmpdir
        best = None
        for it in range(3):
            tmpdir = "/tmp/ntff_profile_bench"
            shutil.rmtree(tmpdir, ignore_errors=True)
            os.makedirs(tmpdir, exist_ok=True)
            res = bass_utils.run_bass_kernel_spmd(
                runner.nc, in_maps, core_ids=list(range(8)), trace=True,
                trace_cores=[0], tmpdir=tmpdir)
            t = res.exec_time_ns
            print(f"bench iter {it}: {t} ns")
            if t is not None and (best is None or t < best):
                best = t
        return best
    except Exception as e:
        print("bench failed:", e)
        return None


def kernel(q, k, v, Wq, Wk, Wv, V_len, Q_len):
    q = np.asarray(q, dtype=np.float32)
    k = np.asarray(k, dtype=np.float32)
    v = np.asarray(v, dtype=np.float32)
    Wq = np.asarray(Wq, dtype=np.float32)
    Wk = np.asarray(Wk, dtype=np.float32)
    Wv = np.asarray(Wv, dtype=np.float32)
    out, _ = _run(q, k, v, Wq, Wk, Wv, V_len, Q_len, bench=False)
    return out


# revision 47
# speedup vs baseline: 1.1654x; 1.0293x over previous
"""Trainium2 Bass kernel for masked multi-head attention (B=4, S=2048, D=512, H=8, dk=64).

Sharding: every batch is split into 8 uniform slices = (head-pair p, k-half kh);
core c = (p = c%4, kh = c//4) runs FOUR slots (one per batch), each with that
batch's EXACT (ceil(nkt/2) k-tiles, nq) shape.  All cores run an identical
instruction stream (SPMD) but per-core work is near-perfectly balanced because
every core owns exactly 1/8 of every batch's score columns.

Per-core kernel tricks:
  - mask-free softmax: the host zeroes k/v columns >= V_len (and k-tile pads),
    so masked scores are exactly 0 -> exp=1, and the matching vp rows AND the
    appended ones-column (host-supplied mask, not memset) are 0, so masked
    positions contribute 0 to both numerator and denominator.  No exp bias.
  - scores computed transposed (S^T[k, q]) feeding AV directly; the two heads
    of a pair run as concurrent K=64 matmuls in PE row-groups 0-1 / 2-3
    (tile_position auto-derived from kp row offsets 0 / 64).
  - V gets a masked ones-column so the AV matmul also emits denominators.
  - k-half partial sums (numerator rows + denominator row) are combined on the
    host before the divide; host also applies the q mask.
  - inputs are host-prefolded to the exact SBUF layout [128, 4*n] so every
    input DMA is 128 fully-contiguous descriptors (HWDGE via the sync queue);
    outputs drain per q-window on the same ring after inputs complete.
"""

import numpy as np
import ml_dtypes

import concourse.bass as bass
import concourse.tile as tile
from concourse import bacc
from concourse import mybir
from concourse import bass2jax

B, S, D = 4, 2048, 512
HEADS, DK = 8, 64
P = 128
DT = D // P  # 4 contraction chunks over D
E = DK + 1   # 64 output dims + denominator row

BF16 = mybir.dt.bfloat16
F32 = mybir.dt.float32

_COMPILE_CACHE = {}


def _cuts(n, w=512):
    return [(s, min(w, n - s)) for s in range(0, n, w)]


def build_bass(key):
    """key: tuple of (KH, nq) per slot, compute order (ascending work)."""
    nc = bacc.Bacc(None, target_bir_lowering=False, debug=False)
    slots = list(key)
    KTS = sum(kh for kh, _ in slots)
    NQS = sum(nq for _, nq in slots)
    NKS = sum(kh * P for kh, _ in slots)
    max_nq = max(nq for _, nq in slots)

    qc = nc.declare_dram_parameter("qc", [P, DT * NQS], BF16, isOutput=False)
    kvc = nc.declare_dram_parameter(
        "kvc", [P, 2 * DT * NKS], BF16, isOutput=False)
    wc = nc.declare_dram_parameter("wc", [P, DT * 3 * P], BF16, isOutput=False)
    om = nc.declare_dram_parameter("om", [P, KTS * 2], F32, isOutput=False)
    out = nc.declare_dram_parameter(
        "out", [len(slots) * 2 * E, max_nq], F32, isOutput=True)

    with tile.TileContext(nc) as tc:
        with (
            tc.tile_pool(name="w", bufs=1) as w_pool,
            tc.tile_pool(name="x", bufs=1) as x_pool,
            tc.tile_pool(name="pr", bufs=1) as pr_pool,
            tc.tile_pool(name="aexp", bufs=3) as a_pool,
            tc.tile_pool(name="osb", bufs=2) as o_pool,
            tc.tile_pool(name="psS", bufs=3, space="PSUM") as psS,
            tc.tile_pool(name="psO", bufs=2, space="PSUM") as psO,
        ):
            w_sb = w_pool.tile([P, DT, 3 * P], BF16, name="w")
            nc.sync.dma_start(
                out=w_sb, in_=wc.rearrange("p (t n) -> p t n", t=DT))
            om_sb = w_pool.tile([P, KTS, 2], F32, name="om")
            nc.sync.dma_start(
                out=om_sb, in_=om.rearrange("p (t n) -> p t n", n=2))

            x_sb = {}
            qoff = koff = 0
            for si, (KH, nq) in enumerate(slots):
                nk = KH * P
                xq = x_pool.tile([P, DT, nq], BF16, tag=f"xq{si}", name=f"xq{si}")
                nc.sync.dma_start(
                    out=xq,
                    in_=qc[:, DT * qoff:DT * (qoff + nq)].rearrange(
                        "p (t n) -> p t n", t=DT))
                xkv = x_pool.tile([P, 2, DT, nk], BF16, tag=f"xkv{si}",
                                  name=f"xkv{si}")
                kvblk = kvc[:, 2 * DT * koff:2 * DT * (koff + nk)].rearrange(
                    "p (u t n) -> p u t n", u=2, t=DT)
                nc.sync.dma_start(out=xkv[:, 0], in_=kvblk[:, 0])
                nc.sync.dma_start(out=xkv[:, 1], in_=kvblk[:, 1])
                x_sb[si] = (xq, xkv[:, 0], xkv[:, 1])
                qoff += nq
                koff += nk

            def ps_tile(width):
                t = psS.tile([P, 1024], F32, tag="s", name="ps")
                return t[:, :width]

            pr = {}

            def emit_proj(si, ktbase):
                KH, nq = slots[si]
                xq, xk, xv = x_sb[si]
                qp = pr_pool.tile([P, nq], BF16, tag=f"qp{si}", name=f"qp{si}")
                for (off, wd) in _cuts(nq):
                    ps = ps_tile(wd)
                    for kd in range(DT):
                        nc.tensor.matmul(
                            ps, lhsT=w_sb[:, kd, 0:P],
                            rhs=xq[:, kd, off:off + wd],
                            start=(kd == 0), stop=(kd == DT - 1))
                    nc.vector.tensor_copy(out=qp[:, off:off + wd], in_=ps)
                kp = pr_pool.tile([P, KH * P], BF16, tag=f"kp{si}", name=f"kp{si}")
                for (off, wd) in _cuts(KH * P):
                    ps = ps_tile(wd)
                    for kd in range(DT):
                        nc.tensor.matmul(
                            ps, lhsT=w_sb[:, kd, P:2 * P],
                            rhs=xk[:, kd, off:off + wd],
                            start=(kd == 0), stop=(kd == DT - 1))
                    nc.vector.tensor_copy(out=kp[:, off:off + wd], in_=ps)
                vp = pr_pool.tile([P, KH, 2, E], BF16, tag=f"vp{si}", name=f"vp{si}")
                for kt in range(KH):
                    ps = ps_tile(2 * DK)
                    for kd in range(DT):
                        nc.tensor.matmul(
                            ps, lhsT=xv[:, kd, kt * P:(kt + 1) * P],
                            rhs=w_sb[:, kd, 2 * P:3 * P],
                            start=(kd == 0), stop=(kd == DT - 1))
                    nc.vector.tensor_copy(
                        out=vp[:, kt, :, :DK],
                        in_=ps.rearrange("p (h d) -> p h d", d=DK))
                nc.vector.tensor_copy(
                    out=vp[:, :, :, DK], in_=om_sb[:, ktbase:ktbase + KH, :])
                pr[si] = (qp, kp, vp)

            def emit_attn(si):
                KH, nq = slots[si]
                qp, kp, vp = pr[si]
                for (qw0, nqw) in _cuts(nq):  # q-windows <= 512
                    avps = [psO.tile([E, 512], F32, tag="av", name="av")
                            for _ in range(2)]
                    # kt blocks at 512-aligned offsets inside a 2-bank tile
                    # (stride `pad`) so no matmul write crosses a PSUM bank.
                    pad = nqw if 512 % nqw == 0 else 512
                    ck = max(1, min(KH, 1024 // pad))
                    kt0 = 0
                    while kt0 < KH:
                        nk_c = min(ck, KH - kt0)
                        pss = [ps_tile(1024).rearrange(
                            "p (c n) -> p c n", n=pad)[:, :nk_c, :nqw]
                            for _ in range(2)]
                        for ktl in range(nk_c):
                            kt = kt0 + ktl
                            for h in range(2):
                                nc.tensor.matmul(
                                    pss[h][:, ktl],
                                    lhsT=kp[DK * h:DK * (h + 1),
                                            kt * P:(kt + 1) * P],
                                    rhs=qp[DK * h:DK * (h + 1), qw0:qw0 + nqw],
                                    start=True, stop=True)
                        a_sb = [None, None]
                        for h in range(2):
                            a_sb[h] = a_pool.tile(
                                [P, 1024], BF16, tag=f"a{h}",
                                name="a").rearrange(
                                    "p (c n) -> p c n", n=pad)[:, :nk_c, :nqw]
                            nc.scalar.activation(
                                out=a_sb[h], in_=pss[h],
                                func=mybir.ActivationFunctionType.Exp,
                                scale=0.125)
                        for ktl in range(nk_c):
                            kt = kt0 + ktl
                            for h in range(2):
                                nc.tensor.matmul(
                                    avps[h][:, :nqw],
                                    lhsT=vp[:, kt, h, :],
                                    rhs=a_sb[h][:, ktl],
                                    start=(kt == 0), stop=(kt == KH - 1))
                        kt0 += nk_c
                    for h in range(2):
                        o_sb = o_pool.tile([E, 512], F32, tag="o", name="o")
                        nc.vector.tensor_copy(
                            out=o_sb[:, :nqw], in_=avps[h][:, :nqw])
                        nc.sync.dma_start(
                            out=out[(si * 2 + h) * E:(si * 2 + h + 1) * E,
                                    qw0:qw0 + nqw],
                            in_=o_sb[:, :nqw])

            # software-pipelined emission: proj(s+1) interleaves ahead of attn(s)
            ktbases = []
            kb = 0
            for (KH, _) in slots:
                ktbases.append(kb)
                kb += KH
            n = len(slots)
            emit_proj(0, ktbases[0])
            for si in range(n):
                # proj(s+1) interleaves ahead of attn(s) — except the LAST
                # slot's projection, whose (late-arriving) inputs must never
                # block the already-fed attn(s) queued behind it in the
                # in-order PE stream.
                if si + 1 < n - 1:
                    emit_proj(si + 1, ktbases[si + 1])
                emit_attn(si)
                if si + 1 == n - 1:
                    emit_proj(si + 1, ktbases[si + 1])
    nc.finalize()
    return nc


class _Runner:
    """Compile the Bass graph once and expose run() over 8 cores."""

    def __init__(self, key, n_cores=8):
        import jax
        from jax.experimental.shard_map import shard_map
        from jax.sharding import Mesh, PartitionSpec

        self.jax = jax
        self.n_cores = n_cores
        nc = build_bass(key)
        self.nc = nc
        bass2jax.install_neuronx_cc_hook()
        assert nc.dbg_addr is None
        partition_name = (
            nc.partition_id_tensor.name if nc.partition_id_tensor else None
        )

        in_names, out_names, out_avals, zero_outs = [], [], [], []
        for alloc in nc.m.functions[0].allocations:
            if not isinstance(alloc, mybir.MemoryLocationSet):
                continue
            name = alloc.memorylocations[0].name
            if alloc.kind == "ExternalInput":
                if name != partition_name:
                    in_names.append(name)
            elif alloc.kind == "ExternalOutput":
                shape = tuple(alloc.tensor_shape)
                dtype = mybir.dt.np(alloc.dtype)
                out_names.append(name)
                out_avals.append(jax.core.ShapedArray(shape, dtype))
                zero_outs.append(np.zeros(shape, dtype))
        self.in_names = list(in_names)
        self.out_names = out_names
        self.zero_outs = zero_outs
        n_params = len(in_names)
        all_names = in_names + out_names
        if partition_name is not None:
            all_names = all_names + [partition_name]

        def _body(*args):
            operands = list(args)
            if partition_name is not None:
                operands.append(bass2jax.partition_id_tensor())
            outs = bass2jax._bass_exec_p.bind(
                *operands,
                out_avals=tuple(out_avals),
                in_names=tuple(all_names),
                out_names=tuple(out_names),
                lowering_input_output_aliases=(),
                sim_require_finite=True,
                sim_require_nnan=True,
                nc=nc,
            )
            return tuple(outs)

        devices = jax.devices()[:n_cores]
        self.mesh = Mesh(np.asarray(devices), ("core",))
        n_outs = len(out_names)
        in_specs = (PartitionSpec("core"),) * (n_params + n_outs)
        out_specs = (PartitionSpec("core"),) * n_outs
        donate = tuple(range(n_params, n_params + n_outs))
        mapped = shard_map(
            _body, mesh=self.mesh, in_specs=in_specs, out_specs=out_specs,
            check_rep=False,
        )
        self._run_jit = jax.jit(mapped, donate_argnums=donate, keep_unused=True)

    def _concat_inputs(self, in_maps):
        per_core = [[np.asarray(m[n]) for n in self.in_names] for m in in_maps]
        concat = [
            np.concatenate([per_core[c][i] for c in range(self.n_cores)], axis=0)
            for i in range(len(self.in_names))
        ]
        concat += [
            np.concatenate([z] * self.n_cores, axis=0) for z in self.zero_outs
        ]
        return concat

    def run(self, in_maps):
        concat = self._concat_inputs(in_maps)
        outs = self._run_jit(*concat)
        results = [{} for _ in range(self.n_cores)]
        for name, arr in zip(self.out_names, outs):
            arr = np.asarray(arr)
            per = np.split(arr, self.n_cores, axis=0)
            for c in range(self.n_cores):
                results[c][name] = per[c]
        return results


def _get_compiled(key, n_cores=8):
    ck = (key, n_cores)
    if ck not in _COMPILE_CACHE:
        _COMPILE_CACHE[ck] = _Runner(key, n_cores=n_cores)
    return _COMPILE_CACHE[ck]


def _plan(V_len, Q_len):
    """Per-batch exact shapes; slots ordered by ascending work."""
    nkt = np.minimum(S // P, (V_len + P - 1) // P).astype(np.int64)
    nq = np.minimum(S, ((Q_len + 63) // 64) * 64).astype(np.int64)
    work = nkt * nq
    # Ascending work: small slots' compute fills the DMA shadow of the
    # later, larger slots' input transfers.
    order = [int(b) for b in np.argsort(work, kind="stable")]
    slots = [(int((nkt[b] + 1) // 2), int(nq[b])) for b in order]
    return tuple(slots), order


def _fold(arr):
    """[512, n] f32/bf16 -> [128, 4*n] matching sbuf [p, t, n] layout."""
    n = arr.shape[1]
    return np.ascontiguousarray(
        arr.reshape(DT, P, n).transpose(1, 0, 2).reshape(P, DT * n))


def _prep_in_maps(q, k, v, Wq, Wk, Wv, V_len, key, order):
    bf = ml_dtypes.bfloat16
    slots = list(key)
    # shared per-batch prep
    qF, kT, vT, omc = {}, {}, {}, {}
    for si, b in enumerate(order):
        KH, nq = slots[si]
        vl = int(V_len[b])
        qF[b] = _fold(np.ascontiguousarray(q[b].T[:, :nq]).astype(bf))
        kk = np.ascontiguousarray(k[b].T).astype(bf)
        vv = np.ascontiguousarray(v[b].T).astype(bf)
        kk[:, vl:] = 0
        vv[:, vl:] = 0
        kT[b], vT[b] = kk, vv
    in_maps = []
    for c in range(8):
        p, kh = c % 4, c // 4
        cols = slice(p * 2 * DK, (p + 1) * 2 * DK)
        wcat = np.concatenate(
            [Wq[:, cols], Wk[:, cols], Wv[:, cols]], axis=1).astype(bf)
        qparts, kvparts, oparts = [], [], []
        for si, b in enumerate(order):
            KH, nq = slots[si]
            nk = KH * P
            k0 = kh * nk
            ksl = np.zeros((D, nk), bf)
            vsl = np.zeros((D, nk), bf)
            avail = max(0, min(S, k0 + nk) - k0)
            if avail:
                ksl[:, :avail] = kT[b][:, k0:k0 + avail]
                vsl[:, :avail] = vT[b][:, k0:k0 + avail]
            qparts.append(qF[b])
            kvparts.append(_fold(ksl))
            kvparts.append(_fold(vsl))
            gk = k0 + np.arange(nk)  # global k index per (kt, lane)
            msk = (gk < int(V_len[b])).astype(np.float32).reshape(KH, P).T
            oparts.append(np.repeat(msk[:, :, None], 2, axis=2).reshape(P, 2 * KH))
        in_maps.append({
            "qc": np.ascontiguousarray(np.concatenate(qparts, axis=1)),
            "kvc": np.ascontiguousarray(np.concatenate(kvparts, axis=1)),
            "wc": _fold(wcat),
            "om": np.ascontiguousarray(np.concatenate(oparts, axis=1)),
        })
    return in_maps


def _postprocess(results, Q_len, key, order):
    slots = list(key)
    O = np.zeros((B, S, HEADS * DK), dtype=np.float32)
    acc = np.zeros((4, len(slots), 2, E, max(nq for _, nq in slots)),
                   dtype=np.float32)
    for c in range(8):
        r = np.asarray(results[c]["out"], dtype=np.float32)
        p, kh = c % 4, c // 4
        for si in range(len(slots)):
            for j in range(2):
                blk = r[(si * 2 + j) * E:(si * 2 + j + 1) * E, :]
                acc[p, si, j, :, :blk.shape[1]] += blk
    for si, b in enumerate(order):
        KH, nq = slots[si]
        ql = min(int(Q_len[b]), nq)
        for p in range(4):
            for j in range(2):
                head = 2 * p + j
                m = acc[p, si, j]
                o = m[:DK, :ql] / m[DK:DK + 1, :ql]
                O[b, :ql, head * DK:(head + 1) * DK] = o.T
    return O


def _run(q, k, v, Wq, Wk, Wv, V_len, Q_len, bench=False):
    V_len = np.asarray(V_len).astype(np.int64)
    Q_len = np.asarray(Q_len).astype(np.int64)
    key, order = _plan(V_len, Q_len)
    runner = _get_compiled(key)
    in_maps = _prep_in_maps(q, k, v, Wq, Wk, Wv, V_len, key, order)
    results = runner.run(in_maps)
    out = _postprocess(results, Q_len, key, order)
    exec_ns = _bench_hw(runner, in_maps) if bench else None
    return out, exec_ns


def _bench_hw(runner, in_maps):
    """NTFF-profiled execution via run_bass_kernel_spmd(trace=True)."""
    import sys
    import types
    import os
    import shutil
    try:
        import trn_agent_boot.trn_boot as tb
        hook = tb._ntff_profile_via_ctypes('/opt/axon/libaxon_pjrt.so')
        if hook is None:
            return None
        if 'antenv.axon_hooks' not in sys.modules:
            m = types.ModuleType('antenv.axon_hooks')
            m.get_axon_ntff_profile_hook = lambda: hook
            sys.modules['antenv.axon_hooks'] = m
        from concourse import bass_utils
        bass_utils.upload_artifacts = lambda tmpdir: "local://" + tmpdir
        best = None
        for it in range(3):
            tmpdir = "/tmp/ntff_profile_bench"
            shutil.rmtree(tmpdir, ignore_errors=True)
            os.makedirs(tmpdir, exist_ok=True)
            res = bass_utils.run_bass_kernel_spmd(
                runner.nc, in_maps, core_ids=list(range(8)), trace=True,
                trace_cores=[0], tmpdir=tmpdir)
            t = res.exec_time_ns
            print(f"bench iter {it}: {t} ns")
            if t is not None and (best is None or t < best):
                best = t
        return best
    except Exception as e:
        print("bench failed:", e)
        return None


def kernel(q, k, v, Wq, Wk, Wv, V_len, Q_len):
    q = np.asarray(q, dtype=np.float32)
    k = np.asarray(k, dtype=np.float32)
    v = np.asarray(v, dtype=np.float32)
    Wq = np.asarray(Wq, dtype=np.float32)
    Wk = np.asarray(Wk, dtype=np.float32)
    Wv = np.asarray(Wv, dtype=np.float32)
    out, _ = _run(q, k, v, Wq, Wk, Wv, V_len, Q_len, bench=False)
    return out


# revision 48
# speedup vs baseline: 1.1683x; 1.0025x over previous
"""Trainium2 Bass kernel for masked multi-head attention (B=4, S=2048, D=512, H=8, dk=64).

Sharding: every batch is split into 8 uniform slices = (head-pair p, k-half kh);
core c = (p = c%4, kh = c//4) runs FOUR slots (one per batch), each with that
batch's EXACT (ceil(nkt/2) k-tiles, nq) shape.  All cores run an identical
instruction stream (SPMD) but per-core work is near-perfectly balanced because
every core owns exactly 1/8 of every batch's score columns.

Per-core kernel tricks:
  - mask-free softmax: the host zeroes k/v columns >= V_len (and k-tile pads),
    so masked scores are exactly 0 -> exp=1, and the matching vp rows AND the
    appended ones-column (host-supplied mask, not memset) are 0, so masked
    positions contribute 0 to both numerator and denominator.  No exp bias.
  - scores computed transposed (S^T[k, q]) feeding AV directly; the two heads
    of a pair run as concurrent K=64 matmuls in PE row-groups 0-1 / 2-3
    (tile_position auto-derived from kp row offsets 0 / 64).
  - V gets a masked ones-column so the AV matmul also emits denominators.
  - k-half partial sums (numerator rows + denominator row) are combined on the
    host before the divide; host also applies the q mask.
  - inputs are host-prefolded to the exact SBUF layout [128, 4*n] so every
    input DMA is 128 fully-contiguous descriptors (HWDGE via the sync queue);
    outputs drain per q-window on the same ring after inputs complete.
"""

import numpy as np
import ml_dtypes

import concourse.bass as bass
import concourse.tile as tile
from concourse import bacc
from concourse import mybir
from concourse import bass2jax

B, S, D = 4, 2048, 512
HEADS, DK = 8, 64
P = 128
DT = D // P  # 4 contraction chunks over D
E = DK + 1   # 64 output dims + denominator row

BF16 = mybir.dt.bfloat16
F32 = mybir.dt.float32

_COMPILE_CACHE = {}


def _cuts(n, w=512):
    return [(s, min(w, n - s)) for s in range(0, n, w)]


def build_bass(key):
    """key: tuple of (KH, nq) per slot, compute order (ascending work)."""
    nc = bacc.Bacc(None, target_bir_lowering=False, debug=False)
    slots = list(key)
    KTS = sum(kh for kh, _ in slots)
    NQS = sum(nq for _, nq in slots)
    NKS = sum(kh * P for kh, _ in slots)
    max_nq = max(nq for _, nq in slots)

    qc = nc.declare_dram_parameter("qc", [P, DT * NQS], BF16, isOutput=False)
    kvc = nc.declare_dram_parameter(
        "kvc", [P, 2 * DT * NKS], BF16, isOutput=False)
    wc = nc.declare_dram_parameter("wc", [P, DT * 3 * P], BF16, isOutput=False)
    om = nc.declare_dram_parameter("om", [P, KTS * 2], F32, isOutput=False)
    out = nc.declare_dram_parameter(
        "out", [len(slots) * 2 * E, max_nq], F32, isOutput=True)

    with tile.TileContext(nc) as tc:
        with (
            tc.tile_pool(name="w", bufs=1) as w_pool,
            tc.tile_pool(name="x", bufs=1) as x_pool,
            tc.tile_pool(name="pr", bufs=1) as pr_pool,
            tc.tile_pool(name="aexp", bufs=3) as a_pool,
            tc.tile_pool(name="osb", bufs=2) as o_pool,
            tc.tile_pool(name="psS", bufs=3, space="PSUM") as psS,
            tc.tile_pool(name="psO", bufs=2, space="PSUM") as psO,
        ):
            w_sb = w_pool.tile([P, DT, 3 * P], BF16, name="w")
            nc.sync.dma_start(
                out=w_sb, in_=wc.rearrange("p (t n) -> p t n", t=DT))
            om_sb = w_pool.tile([P, KTS, 2], F32, name="om")
            nc.sync.dma_start(
                out=om_sb, in_=om.rearrange("p (t n) -> p t n", n=2))

            x_sb = {}
            qoff = koff = 0
            for si, (KH, nq) in enumerate(slots):
                nk = KH * P
                xq = x_pool.tile([P, DT, nq], BF16, tag=f"xq{si}", name=f"xq{si}")
                nc.sync.dma_start(
                    out=xq,
                    in_=qc[:, DT * qoff:DT * (qoff + nq)].rearrange(
                        "p (t n) -> p t n", t=DT))
                xkv = x_pool.tile([P, 2, DT, nk], BF16, tag=f"xkv{si}",
                                  name=f"xkv{si}")
                kvblk = kvc[:, 2 * DT * koff:2 * DT * (koff + nk)].rearrange(
                    "p (u t n) -> p u t n", u=2, t=DT)
                nc.sync.dma_start(out=xkv[:, 0], in_=kvblk[:, 0])
                nc.sync.dma_start(out=xkv[:, 1], in_=kvblk[:, 1])
                x_sb[si] = (xq, xkv[:, 0], xkv[:, 1])
                qoff += nq
                koff += nk

            def ps_tile(width):
                t = psS.tile([P, 1024], F32, tag="s", name="ps")
                return t[:, :width]

            pr = {}

            def emit_proj(si, ktbase):
                KH, nq = slots[si]
                xq, xk, xv = x_sb[si]
                qp = pr_pool.tile([P, nq], BF16, tag=f"qp{si}", name=f"qp{si}")
                for (off, wd) in _cuts(nq):
                    ps = ps_tile(wd)
                    for kd in range(DT):
                        nc.tensor.matmul(
                            ps, lhsT=w_sb[:, kd, 0:P],
                            rhs=xq[:, kd, off:off + wd],
                            start=(kd == 0), stop=(kd == DT - 1))
                    nc.vector.tensor_copy(out=qp[:, off:off + wd], in_=ps)
                kp = pr_pool.tile([P, KH * P], BF16, tag=f"kp{si}", name=f"kp{si}")
                for (off, wd) in _cuts(KH * P):
                    ps = ps_tile(wd)
                    for kd in range(DT):
                        nc.tensor.matmul(
                            ps, lhsT=w_sb[:, kd, P:2 * P],
                            rhs=xk[:, kd, off:off + wd],
                            start=(kd == 0), stop=(kd == DT - 1))
                    nc.vector.tensor_copy(out=kp[:, off:off + wd], in_=ps)
                vp = pr_pool.tile([P, KH, 2, E], BF16, tag=f"vp{si}", name=f"vp{si}")
                for kt in range(KH):
                    ps = ps_tile(2 * DK)
                    for kd in range(DT):
                        nc.tensor.matmul(
                            ps, lhsT=xv[:, kd, kt * P:(kt + 1) * P],
                            rhs=w_sb[:, kd, 2 * P:3 * P],
                            start=(kd == 0), stop=(kd == DT - 1))
                    nc.vector.tensor_copy(
                        out=vp[:, kt, :, :DK],
                        in_=ps.rearrange("p (h d) -> p h d", d=DK))
                nc.vector.tensor_copy(
                    out=vp[:, :, :, DK], in_=om_sb[:, ktbase:ktbase + KH, :])
                pr[si] = (qp, kp, vp)

            def emit_attn(si):
                KH, nq = slots[si]
                qp, kp, vp = pr[si]
                for (qw0, nqw) in _cuts(nq):  # q-windows <= 512
                    avps = [psO.tile([E, 512], F32, tag="av", name="av")
                            for _ in range(2)]
                    # kt blocks at 512-aligned offsets inside a 2-bank tile
                    # (stride `pad`) so no matmul write crosses a PSUM bank.
                    pad = nqw if 512 % nqw == 0 else 512
                    ck = max(1, min(KH, 1024 // pad))
                    kt0 = 0
                    while kt0 < KH:
                        nk_c = min(ck, KH - kt0)
                        pss = [ps_tile(1024).rearrange(
                            "p (c n) -> p c n", n=pad)[:, :nk_c, :nqw]
                            for _ in range(2)]
                        for ktl in range(nk_c):
                            kt = kt0 + ktl
                            for h in range(2):
                                nc.tensor.matmul(
                                    pss[h][:, ktl],
                                    lhsT=kp[DK * h:DK * (h + 1),
                                            kt * P:(kt + 1) * P],
                                    rhs=qp[DK * h:DK * (h + 1), qw0:qw0 + nqw],
                                    start=True, stop=True)
                        a_sb = [None, None]
                        for h in range(2):
                            a_sb[h] = a_pool.tile(
                                [P, 1024], BF16, tag=f"a{h}",
                                name="a").rearrange(
                                    "p (c n) -> p c n", n=pad)[:, :nk_c, :nqw]
                            nc.scalar.activation(
                                out=a_sb[h], in_=pss[h],
                                func=mybir.ActivationFunctionType.Exp,
                                scale=0.125)
                        for ktl in range(nk_c):
                            kt = kt0 + ktl
                            for h in range(2):
                                nc.tensor.matmul(
                                    avps[h][:, :nqw],
                                    lhsT=vp[:, kt, h, :],
                                    rhs=a_sb[h][:, ktl],
                                    start=(kt == 0), stop=(kt == KH - 1))
                        kt0 += nk_c
                    for h in range(2):
                        o_sb = o_pool.tile([E, 512], F32, tag="o", name="o")
                        nc.vector.tensor_copy(
                            out=o_sb[:, :nqw], in_=avps[h][:, :nqw])
                        nc.sync.dma_start(
                            out=out[(si * 2 + h) * E:(si * 2 + h + 1) * E,
                                    qw0:qw0 + nqw],
                            in_=o_sb[:, :nqw])

            # software-pipelined emission: proj(s+1) interleaves ahead of attn(s)
            ktbases = []
            kb = 0
            for (KH, _) in slots:
                ktbases.append(kb)
                kb += KH
            n = len(slots)
            # Strict per-slot order: each slot's (possibly late-arriving)
            # projection inputs must never block already-fed attention work
            # queued behind them in the in-order PE stream.
            for si in range(n):
                emit_proj(si, ktbases[si])
                emit_attn(si)
    nc.finalize()
    return nc


class _Runner:
    """Compile the Bass graph once and expose run() over 8 cores."""

    def __init__(self, key, n_cores=8):
        import jax
        from jax.experimental.shard_map import shard_map
        from jax.sharding import Mesh, PartitionSpec

        self.jax = jax
        self.n_cores = n_cores
        nc = build_bass(key)
        self.nc = nc
        bass2jax.install_neuronx_cc_hook()
        assert nc.dbg_addr is None
        partition_name = (
            nc.partition_id_tensor.name if nc.partition_id_tensor else None
        )

        in_names, out_names, out_avals, zero_outs = [], [], [], []
        for alloc in nc.m.functions[0].allocations:
            if not isinstance(alloc, mybir.MemoryLocationSet):
                continue
            name = alloc.memorylocations[0].name
            if alloc.kind == "ExternalInput":
                if name != partition_name:
                    in_names.append(name)
            elif alloc.kind == "ExternalOutput":
                shape = tuple(alloc.tensor_shape)
                dtype = mybir.dt.np(alloc.dtype)
                out_names.append(name)
                out_avals.append(jax.core.ShapedArray(shape, dtype))
                zero_outs.append(np.zeros(shape, dtype))
        self.in_names = list(in_names)
        self.out_names = out_names
        self.zero_outs = zero_outs
        n_params = len(in_names)
        all_names = in_names + out_names
        if partition_name is not None:
            all_names = all_names + [partition_name]

        def _body(*args):
            operands = list(args)
            if partition_name is not None:
                operands.append(bass2jax.partition_id_tensor())
            outs = bass2jax._bass_exec_p.bind(
                *operands,
                out_avals=tuple(out_avals),
                in_names=tuple(all_names),
                out_names=tuple(out_names),
                lowering_input_output_aliases=(),
                sim_require_finite=True,
                sim_require_nnan=True,
                nc=nc,
            )
            return tuple(outs)

        devices = jax.devices()[:n_cores]
        self.mesh = Mesh(np.asarray(devices), ("core",))
        n_outs = len(out_names)
        in_specs = (PartitionSpec("core"),) * (n_params + n_outs)
        out_specs = (PartitionSpec("core"),) * n_outs
        donate = tuple(range(n_params, n_params + n_outs))
        mapped = shard_map(
            _body, mesh=self.mesh, in_specs=in_specs, out_specs=out_specs,
            check_rep=False,
        )
        self._run_jit = jax.jit(mapped, donate_argnums=donate, keep_unused=True)

    def _concat_inputs(self, in_maps):
        per_core = [[np.asarray(m[n]) for n in self.in_names] for m in in_maps]
        concat = [
            np.concatenate([per_core[c][i] for c in range(self.n_cores)], axis=0)
            for i in range(len(self.in_names))
        ]
        concat += [
            np.concatenate([z] * self.n_cores, axis=0) for z in self.zero_outs
        ]
        return concat

    def run(self, in_maps):
        concat = self._concat_inputs(in_maps)
        outs = self._run_jit(*concat)
        results = [{} for _ in range(self.n_cores)]
        for name, arr in zip(self.out_names, outs):
            arr = np.asarray(arr)
            per = np.split(arr, self.n_cores, axis=0)
            for c in range(self.n_cores):
                results[c][name] = per[c]
        return results


def _get_compiled(key, n_cores=8):
    ck = (key, n_cores)
    if ck not in _COMPILE_CACHE:
        _COMPILE_CACHE[ck] = _Runner(key, n_cores=n_cores)
    return _COMPILE_CACHE[ck]


def _plan(V_len, Q_len):
    """Per-batch exact shapes; slots ordered by ascending work."""
    nkt = np.minimum(S // P, (V_len + P - 1) // P).astype(np.int64)
    nq = np.minimum(S, ((Q_len + 63) // 64) * 64).astype(np.int64)
    work = nkt * nq
    # Ascending work: small slots' compute fills the DMA shadow of the
    # later, larger slots' input transfers.
    order = [int(b) for b in np.argsort(work, kind="stable")]
    slots = [(int((nkt[b] + 1) // 2), int(nq[b])) for b in order]
    return tuple(slots), order


def _fold(arr):
    """[512, n] f32/bf16 -> [128, 4*n] matching sbuf [p, t, n] layout."""
    n = arr.shape[1]
    return np.ascontiguousarray(
        arr.reshape(DT, P, n).transpose(1, 0, 2).reshape(P, DT * n))


def _prep_in_maps(q, k, v, Wq, Wk, Wv, V_len, key, order):
    bf = ml_dtypes.bfloat16
    slots = list(key)
    # shared per-batch prep
    qF, kT, vT, omc = {}, {}, {}, {}
    for si, b in enumerate(order):
        KH, nq = slots[si]
        vl = int(V_len[b])
        qF[b] = _fold(np.ascontiguousarray(q[b].T[:, :nq]).astype(bf))
        kk = np.ascontiguousarray(k[b].T).astype(bf)
        vv = np.ascontiguousarray(v[b].T).astype(bf)
        kk[:, vl:] = 0
        vv[:, vl:] = 0
        kT[b], vT[b] = kk, vv
    in_maps = []
    for c in range(8):
        p, kh = c % 4, c // 4
        cols = slice(p * 2 * DK, (p + 1) * 2 * DK)
        wcat = np.concatenate(
            [Wq[:, cols], Wk[:, cols], Wv[:, cols]], axis=1).astype(bf)
        qparts, kvparts, oparts = [], [], []
        for si, b in enumerate(order):
            KH, nq = slots[si]
            nk = KH * P
            k0 = kh * nk
            ksl = np.zeros((D, nk), bf)
            vsl = np.zeros((D, nk), bf)
            avail = max(0, min(S, k0 + nk) - k0)
            if avail:
                ksl[:, :avail] = kT[b][:, k0:k0 + avail]
                vsl[:, :avail] = vT[b][:, k0:k0 + avail]
            qparts.append(qF[b])
            kvparts.append(_fold(ksl))
            kvparts.append(_fold(vsl))
            gk = k0 + np.arange(nk)  # global k index per (kt, lane)
            msk = (gk < int(V_len[b])).astype(np.float32).reshape(KH, P).T
            oparts.append(np.repeat(msk[:, :, None], 2, axis=2).reshape(P, 2 * KH))
        in_maps.append({
            "qc": np.ascontiguousarray(np.concatenate(qparts, axis=1)),
            "kvc": np.ascontiguousarray(np.concatenate(kvparts, axis=1)),
            "wc": _fold(wcat),
            "om": np.ascontiguousarray(np.concatenate(oparts, axis=1)),
        })
    return in_maps


def _postprocess(results, Q_len, key, order):
    slots = list(key)
    O = np.zeros((B, S, HEADS * DK), dtype=np.float32)
    acc = np.zeros((4, len(slots), 2, E, max(nq for _, nq in slots)),
                   dtype=np.float32)
    for c in range(8):
        r = np.asarray(results[c]["out"], dtype=np.float32)
        p, kh = c % 4, c // 4
        for si in range(len(slots)):
            for j in range(2):
                blk = r[(si * 2 + j) * E:(si * 2 + j + 1) * E, :]
                acc[p, si, j, :, :blk.shape[1]] += blk
    for si, b in enumerate(order):
        KH, nq = slots[si]
        ql = min(int(Q_len[b]), nq)
        for p in range(4):
            for j in range(2):
                head = 2 * p + j
                m = acc[p, si, j]
                o = m[:DK, :ql] / m[DK:DK + 1, :ql]
                O[b, :ql, head * DK:(head + 1) * DK] = o.T
    return O


def _run(q, k, v, Wq, Wk, Wv, V_len, Q_len, bench=False):
    V_len = np.asarray(V_len).astype(np.int64)
    Q_len = np.asarray(Q_len).astype(np.int64)
    key, order = _plan(V_len, Q_len)
    runner = _get_compiled(key)
    in_maps = _prep_in_maps(q, k, v, Wq, Wk, Wv, V_len, key, order)
    results = runner.run(in_maps)
    out = _postprocess(results, Q_len, key, order)
    exec_ns = _bench_hw(runner, in_maps) if bench else None
    return out, exec_ns


def _bench_hw(runner, in_maps):
    """NTFF-profiled execution via run_bass_kernel_spmd(trace=True)."""
    import sys
    import types
    import os
    import shutil
    try:
        import trn_agent_boot.trn_boot as tb
        hook = tb._ntff_profile_via_ctypes('/opt/axon/libaxon_pjrt.so')
        if hook is None:
            return None
        if 'antenv.axon_hooks' not in sys.modules:
            m = types.ModuleType('antenv.axon_hooks')
            m.get_axon_ntff_profile_hook = lambda: hook
            sys.modules['antenv.axon_hooks'] = m
        from concourse import bass_utils
        bass_utils.upload_artifacts = lambda tmpdir: "local://" + tmpdir
        best = None
        for it in range(3):
            tmpdir = "/tmp/ntff_profile_bench"
            shutil.rmtree(tmpdir, ignore_errors=True)
            os.makedirs(tmpdir, exist_ok=True)
            res = bass_utils.run_bass_kernel_spmd(
                runner.nc, in_maps, core_ids=list(range(8)), trace=True,
                trace_cores=[0], tmpdir=tmpdir)
            t = res.exec_time_ns
            print(f"bench iter {it}: {t} ns")
            if t is not None and (best is None or t < best):
                best = t
        return best
    except Exception as e:
        print("bench failed:", e)
        return None


def kernel(q, k, v, Wq, Wk, Wv, V_len, Q_len):
    q = np.asarray(q, dtype=np.float32)
    k = np.asarray(k, dtype=np.float32)
    v = np.asarray(v, dtype=np.float32)
    Wq = np.asarray(Wq, dtype=np.float32)
    Wk = np.asarray(Wk, dtype=np.float32)
    Wv = np.asarray(Wv, dtype=np.float32)
    out, _ = _run(q, k, v, Wq, Wk, Wv, V_len, Q_len, bench=False)
    return out
